# revision 8
# baseline (speedup 1.0000x reference)
"""Causal self-attention on 8 Trainium2 NeuronCores.

Sharding: data-parallel over batch (B=2) x tensor-parallel over head groups
(16 heads -> 4 groups of 4). Core (b, g) computes, for batch b and heads
4g..4g+3: the qkv projection slice, causal attention, and a partial output
projection O_partial = Y_g @ W_proj[rows_g]. The host sums the 4 partials per
batch and adds b_proj.

Per-core kernel (all matmuls in float32r = TF32-like, full PE rate):
  phase j = q-tile of 512 tokens:
    QKV(j):  Q^T/K^T per head-pair ([128,512] psum tiles, contraction over
             8 c-chunks of x^T), V in [t,d] layout with a ones column
             appended per head (gives the softmax denominator for free).
    ATTN(j): per head pair, per k-chunk-pair: S^T = K^T.T @ Q^T ->
             exp(S/8) (ScalarE, psum->sbuf) -> causal mask multiply on
             diagonal chunks -> Y^T accumulation = [V|1].T @ expS.
             Softmax denominator lands in row 64 of the Y psum tile;
             normalize via reciprocal + partition-broadcast + multiply.
    PROJ(j): O rows = Y^T.T @ W_proj slice, accumulated over the 2
             128-channel chunks, copied to SBUF and DMA'd out.
  QKV(j+1) and PROJ(j-1) units are interleaved between attention steps so
  the PE stays busy while ScalarE works through the exps.
"""

import numpy as np

B, T, C, H, D = 2, 2048, 1024, 16, 64
NCORES = 8
GQ = 4            # head groups (tensor parallel)
HPC = H // GQ     # heads per core = 4
CS = HPC * D      # per-core channel slice = 256
NJ = 4            # q tiles
QT = 512          # q tile size
NKC = 16          # k chunks of 128

_programs = {}
LAST_RESULTS = None


def _build_program(bench_iters=None):
    import concourse.bacc as bacc_mod
    import concourse.tile as tile
    import concourse.mybir as mybir
    from contextlib import ExitStack
    from collections import deque

    f32 = mybir.dt.float32
    f32r = mybir.dt.float32r
    Act = mybir.ActivationFunctionType

    nc = bacc_mod.Bacc(
        "TRN2", target_bir_lowering=False, debug=False, num_devices=NCORES
    )

    x_T = nc.dram_tensor("x_T", [C, T], f32r, kind="ExternalInput")
    w_q = nc.dram_tensor("w_q", [C, CS], f32r, kind="ExternalInput")
    w_k = nc.dram_tensor("w_k", [C, CS], f32r, kind="ExternalInput")
    w_v = nc.dram_tensor("w_v", [C, CS], f32r, kind="ExternalInput")
    w_p = nc.dram_tensor("w_p", [CS, C], f32r, kind="ExternalInput")
    b_q = nc.dram_tensor("b_q", [128, 2], f32, kind="ExternalInput")
    b_k = nc.dram_tensor("b_k", [128, 2], f32, kind="ExternalInput")
    b_v = nc.dram_tensor("b_v", [1, CS], f32, kind="ExternalInput")
    msk = nc.dram_tensor("msk", [128, 512], f32r, kind="ExternalInput")
    vones = nc.dram_tensor("vones", [128, NKC], f32r, kind="ExternalInput")
    o = nc.dram_tensor("o", [T, C], f32, kind="ExternalOutput")

    with tile.TileContext(nc) as tc:
        with ExitStack() as ctx:
            const = ctx.enter_context(tc.tile_pool(name="const", bufs=1))
            qpool = ctx.enter_context(tc.tile_pool(name="qpool", bufs=2))
            ypool = ctx.enter_context(tc.tile_pool(name="ypool", bufs=2))
            epool = ctx.enter_context(tc.tile_pool(name="epool", bufs=3))
            npool = ctx.enter_context(tc.tile_pool(name="npool", bufs=1))
            opool = ctx.enter_context(tc.tile_pool(name="opool", bufs=2))
            pspool = ctx.enter_context(
                tc.tile_pool(name="pspool", bufs=1, space="PSUM")
            )

            # ---- persistent tiles ----
            wq_sb = const.tile([128, 8, CS], f32r, name="wq_sb")
            wk_sb = const.tile([128, 8, CS], f32r, name="wk_sb")
            wv_sb = const.tile([128, 8, CS], f32r, name="wv_sb")
            wp_sb = const.tile([128, 2, C], f32r, name="wp_sb")
            msk_sb = const.tile([128, 512], f32r, name="msk_sb")
            bq_sb = const.tile([128, 2], f32, name="bq_sb")
            bk_sb = const.tile([128, 2], f32, name="bk_sb")
            bv_row = const.tile([1, CS], f32, name="bv_row")
            bv_sb = const.tile([128, CS], f32, name="bv_sb")
            x_sb = [
                const.tile([128, T], f32r, name=f"x_sb{n}", tag=f"x_sb{n}")
                for n in range(8)
            ]
            kT_sb = const.tile([128, 2, T], f32r, name="kT_sb")
            v_sb = const.tile([128, NKC, HPC * 65], f32r, name="v_sb")

            def body(tag=""):
                # ---- constant/weight/input DMAs ----
                nc.sync.dma_start(
                    wq_sb[:], w_q[:].rearrange("(n p) d -> p n d", p=128)
                )
                nc.sync.dma_start(
                    wk_sb[:], w_k[:].rearrange("(n p) d -> p n d", p=128)
                )
                nc.sync.dma_start(
                    wv_sb[:], w_v[:].rearrange("(n p) d -> p n d", p=128)
                )
                nc.sync.dma_start(
                    wp_sb[:], w_p[:].rearrange("(n p) d -> p n d", p=128)
                )
                nc.sync.dma_start(msk_sb[:], msk[:])
                nc.sync.dma_start(bq_sb[:], b_q[:])
                nc.sync.dma_start(bk_sb[:], b_k[:])
                nc.sync.dma_start(bv_row[:], b_v[:])
                nc.gpsimd.partition_broadcast(bv_sb[:], bv_row[0:1, :])
                for n in range(8):
                    nc.sync.dma_start(x_sb[n][:], x_T[128 * n : 128 * (n + 1), :])
                for hl in range(HPC):
                    nc.sync.dma_start(
                        v_sb[:, :, 65 * hl + 64 : 65 * hl + 65],
                        vones[:].rearrange("p (a u) -> p a u", u=1),
                    )

                qT = {}   # j -> tile [128, 2, QT]
                yT = {}   # j -> tile [128, 2, QT]

                # ---- work units ----
                def unit_q(j, p):
                    def emit():
                        ps_t = pspool.tile(
                            [128, QT], f32, tag="acc", bufs=4, name=f"psq{tag}{j}{p}"
                        )
                        for n in range(8):
                            nc.tensor.matmul(
                                ps_t[:],
                                wq_sb[:, n, 128 * p : 128 * (p + 1)],
                                x_sb[n][:, QT * j : QT * (j + 1)],
                                start=(n == 0),
                                stop=(n == 7),
                            )
                        nc.vector.tensor_scalar_add(
                            qT[j][:, p, :], ps_t[:], bq_sb[:, p : p + 1]
                        )
                    return emit

                def unit_k(j, p):
                    def emit():
                        ps_t = pspool.tile(
                            [128, QT], f32, tag="acc", bufs=4, name=f"psk{tag}{j}{p}"
                        )
                        for n in range(8):
                            nc.tensor.matmul(
                                ps_t[:],
                                wk_sb[:, n, 128 * p : 128 * (p + 1)],
                                x_sb[n][:, QT * j : QT * (j + 1)],
                                start=(n == 0),
                                stop=(n == 7),
                            )
                        nc.vector.tensor_scalar_add(
                            kT_sb[:, p, QT * j : QT * (j + 1)],
                            ps_t[:],
                            bk_sb[:, p : p + 1],
                        )
                    return emit

                def unit_v(j, tt):
                    def emit():
                        toff = QT * j + 128 * tt
                        ps_t = pspool.tile(
                            [128, CS], f32, tag="acc", bufs=4, name=f"psv{tag}{j}{tt}"
                        )
                        for n in range(8):
                            nc.tensor.matmul(
                                ps_t[:],
                                x_sb[n][:, toff : toff + 128],
                                wv_sb[:, n, :],
                                start=(n == 0),
                                stop=(n == 7),
                            )
                        kc = 4 * j + tt
                        nc.vector.tensor_add(
                            v_sb[:, kc, :]
                            .rearrange("p (h u) -> p h u", h=HPC)[:, :, 0:64],
                            ps_t[:].rearrange("p (h u) -> p h u", h=HPC),
                            bv_sb[:].rearrange("p (h u) -> p h u", h=HPC),
                        )
                    return emit

                def unit_proj(j, tt, f2):
                    def emit():
                        ps_t = pspool.tile(
                            [128, QT], f32, tag="acc", bufs=4,
                            name=f"pso{tag}{j}{tt}{f2}",
                        )
                        for cp in range(2):
                            nc.tensor.matmul(
                                ps_t[:],
                                yT[j][:, cp, 128 * tt : 128 * (tt + 1)],
                                wp_sb[:, cp, QT * f2 : QT * (f2 + 1)],
                                start=(cp == 0),
                                stop=(cp == 1),
                            )
                        ob = opool.tile(
                            [128, QT], f32, tag="ob", name=f"ob{tag}{j}{tt}{f2}"
                        )
                        nc.vector.tensor_copy(ob[:], ps_t[:])
                        nc.sync.dma_start(
                            o[
                                QT * j + 128 * tt : QT * j + 128 * (tt + 1),
                                QT * f2 : QT * (f2 + 1),
                            ],
                            ob[:],
                        )
                    return emit

                def qkv_units(j):
                    us = []
                    for p in range(2):
                        us.append(unit_q(j, p))
                        us.append(unit_k(j, p))
                    for tt in range(4):
                        us.append(unit_v(j, tt))
                    return us

                def proj_units(j):
                    return [
                        unit_proj(j, tt, f2) for tt in range(4) for f2 in range(2)
                    ]

                def alloc_qT(j):
                    qT[j] = qpool.tile(
                        [128, 2, QT], f32r, tag="qT", name=f"qT{tag}{j}"
                    )

                def alloc_yT(j):
                    yT[j] = ypool.tile(
                        [128, 2, QT], f32r, tag="yT", name=f"yT{tag}{j}"
                    )

                # ---- attention for one q-tile with filler interleave ----
                def attention(j, filler):
                    nkc = 4 * j + 4           # k chunks for this q tile
                    ncp = 2 * j + 2           # chunk pairs
                    alloc_yT(j)
                    total_cps = 2 * ncp
                    cps_done = 0
                    for p in range(2):
                        y0 = pspool.tile(
                            [128, QT], f32, tag="acc", bufs=4, name=f"y0{tag}{j}{p}"
                        )
                        y1 = pspool.tile(
                            [128, QT], f32, tag="acc", bufs=4, name=f"y1{tag}{j}{p}"
                        )
                        for cp in range(ncp):
                            s0 = pspool.tile(
                                [128, 2, QT], f32, tag="s2", bufs=2,
                                name=f"s0{tag}{j}{p}{cp}",
                            )
                            s1 = pspool.tile(
                                [128, 2, QT], f32, tag="s2", bufs=2,
                                name=f"s1{tag}{j}{p}{cp}",
                            )
                            for half in range(2):
                                kc = 2 * cp + half
                                nc.tensor.matmul(
                                    s0[:, half, :],
                                    kT_sb[0:64, p, 128 * kc : 128 * (kc + 1)],
                                    qT[j][0:64, p, :],
                                    start=True,
                                    stop=True,
                                )
                                nc.tensor.matmul(
                                    s1[:, half, :],
                                    kT_sb[64:128, p, 128 * kc : 128 * (kc + 1)],
                                    qT[j][64:128, p, :],
                                    start=True,
                                    stop=True,
                                )
                            e0 = epool.tile(
                                [128, 2, QT], f32r, tag="e2",
                                name=f"e0{tag}{j}{p}{cp}",
                            )
                            e1 = epool.tile(
                                [128, 2, QT], f32r, tag="e2",
                                name=f"e1{tag}{j}{p}{cp}",
                            )
                            nc.scalar.activation(e0[:], s0[:], Act.Exp, scale=0.125)
                            nc.scalar.activation(e1[:], s1[:], Act.Exp, scale=0.125)
                            for half in range(2):
                                kc = 2 * cp + half
                                m = kc - 4 * j
                                if m >= 0:
                                    w = 128 * (m + 1)
                                    nc.vector.tensor_mul(
                                        e0[:, half, 0:w],
                                        e0[:, half, 0:w],
                                        msk_sb[:, 512 - w : 512],
                                    )
                                    nc.vector.tensor_mul(
                                        e1[:, half, 0:w],
                                        e1[:, half, 0:w],
                                        msk_sb[:, 512 - w : 512],
                                    )
                            # filler between S and PV so the PE has work
                            # while ScalarE computes the exps
                            cps_done += 1
                            remaining = total_cps - cps_done
                            if filler:
                                npop = -(-len(filler) // (remaining + 1))
                                for _ in range(npop):
                                    if filler:
                                        filler.popleft()()
                            for half in range(2):
                                kc = 2 * cp + half
                                h0 = 2 * p
                                h1 = 2 * p + 1
                                nc.tensor.matmul(
                                    y0[0:65, :],
                                    v_sb[:, kc, 65 * h0 : 65 * h0 + 65],
                                    e0[:, half, :],
                                    start=(kc == 0),
                                    stop=(kc == nkc - 1),
                                )
                                nc.tensor.matmul(
                                    y1[0:65, :],
                                    v_sb[:, kc, 65 * h1 : 65 * h1 + 65],
                                    e1[:, half, :],
                                    start=(kc == 0),
                                    stop=(kc == nkc - 1),
                                )
                        # normalize the pair: denominators are row 64
                        pk = npool.tile(
                            [1, 2, QT], f32, tag="pk", name=f"pk{tag}{j}{p}"
                        )
                        nc.vector.tensor_copy(pk[0:1, 0, :], y0[64:65, :])
                        nc.vector.tensor_copy(pk[0:1, 1, :], y1[64:65, :])
                        scr = npool.tile(
                            [1, 2, QT], f32, tag="scr", name=f"scr{tag}{j}{p}"
                        )
                        rcp = npool.tile(
                            [1, 2, QT], f32, tag="rcp", name=f"rcp{tag}{j}{p}"
                        )
                        nc.vector.reciprocal_approx_accurate(
                            rcp[:], pk[:], scr[:]
                        )
                        rb0 = npool.tile(
                            [64, QT], f32, tag="rb0", bufs=2, name=f"rb0{tag}{j}{p}"
                        )
                        rb1 = npool.tile(
                            [64, QT], f32, tag="rb1", bufs=2, name=f"rb1{tag}{j}{p}"
                        )
                        nc.gpsimd.partition_broadcast(rb0[:], rcp[0:1, 0, :])
                        nc.gpsimd.partition_broadcast(rb1[:], rcp[0:1, 1, :])
                        nc.vector.tensor_mul(
                            yT[j][0:64, p, :], y0[0:64, :], rb0[:]
                        )
                        nc.vector.tensor_mul(
                            yT[j][64:128, p, :], y1[0:64, :], rb1[:]
                        )

                # ---- main schedule ----
                alloc_qT(0)
                for u in qkv_units(0):
                    u()
                for j in range(NJ):
                    filler = deque()
                    if j + 1 < NJ:
                        alloc_qT(j + 1)
                        filler.extend(qkv_units(j + 1))
                    if j >= 1:
                        filler.extend(proj_units(j - 1))
                    attention(j, filler)
                    while filler:
                        filler.popleft()()
                for u in proj_units(NJ - 1):
                    u()

            if bench_iters is None:
                body()
            else:
                with tc.For_i(0, bench_iters, 1):
                    body()

    nc.compile()
    return nc


def _get_program(bench_iters=None):
    key = bench_iters
    if key not in _programs:
        _programs[key] = _build_program(bench_iters)
    return _programs[key]


def _make_mask():
    # msk[k, v] = 1.0 iff v >= k + 384; slice [:, 512-w:] gives the causal
    # mask for a diagonal chunk with offset m where w = 128*(m+1)
    return np.greater_equal(
        np.arange(512, dtype=np.int32)[None, :],
        np.arange(128, dtype=np.int32)[:, None] + 384,
    ).astype(np.float32)


def _make_in_maps(x, W_qkv, b_qkv, W_proj):
    msk = _make_mask()
    in_maps = []
    for core in range(NCORES):
        b, g = divmod(core, GQ)
        base = CS * g
        in_maps.append({
            "x_T": np.ascontiguousarray(x[b].T),
            "w_q": np.ascontiguousarray(W_qkv[:, base : base + CS]),
            "w_k": np.ascontiguousarray(W_qkv[:, C + base : C + base + CS]),
            "w_v": np.ascontiguousarray(W_qkv[:, 2 * C + base : 2 * C + base + CS]),
            "w_p": np.ascontiguousarray(W_proj[base : base + CS, :]),
            "b_q": np.ascontiguousarray(b_qkv[base : base + CS].reshape(2, 128).T),
            "b_k": np.ascontiguousarray(
                b_qkv[C + base : C + base + CS].reshape(2, 128).T
            ),
            "b_v": b_qkv[2 * C + base : 2 * C + base + CS].reshape(1, CS).copy(),
            "msk": msk,
            "vones": np.ones((128, NKC), np.float32),
        })
    return in_maps


def _run_spmd(nc, in_maps):
    from concourse.bass_utils import run_bass_kernel_spmd

    try:
        return run_bass_kernel_spmd(
            nc, in_maps, core_ids=list(range(NCORES)), trace=False
        )
    except Exception:
        # transient NRT flakes happen; retry once
        return run_bass_kernel_spmd(
            nc, in_maps, core_ids=list(range(NCORES)), trace=False
        )


def kernel(**inputs):
    global LAST_RESULTS
    x = np.asarray(inputs["x"], np.float32)
    W_qkv = np.asarray(inputs["W_qkv"], np.float32)
    b_qkv = np.asarray(inputs["b_qkv"], np.float32)
    W_proj = np.asarray(inputs["W_proj"], np.float32)
    b_proj = np.asarray(inputs["b_proj"], np.float32)

    nc = _get_program(None)
    res = _run_spmd(nc, _make_in_maps(x, W_qkv, b_qkv, W_proj))
    LAST_RESULTS = res

    out = np.zeros((B, T, C), np.float32)
    for core in range(NCORES):
        out[core // GQ] += res.results[core]["o"]
    out += b_proj[None, None, :]
    return out
